# revision 3
# baseline (speedup 1.0000x reference)
"""Trainium2 Bass/Tile kernel for the GatedNode2Edge op.

Computes, for emb (B,C,N), th12_* (E,C), th5_* (E,):
    t_k  = th12_k @ emb[b]                      (E,N)
    m_k  = max(t_k[:,i], t_k[:,j]) pairwise     (E,N,N)
    adj  = relu(2*m_1 + th5_1*I)
    gate = sigmoid(relu(2*m_2 + th5_2*I))
    out  = adj * gate                           (B,E,N,N)

Sharding: the 64 (b,e) channels are split 8-per-core across 8 NeuronCores.
Per channel the output is symmetric (out == out.T), so each core computes
only the upper-triangular 128-row blocks with DVE ops and produces the
lower blocks by PE transpose of the finished upper blocks.

Key per-tile pipeline (row block r of channel ch, W = N - 128*r):
    m2 = tensor_scalar_max(u_jrep[:, 128r:], u_i)          # DVE, 2x fp32
    g  = sigmoid(m2)                                       # ACT
    o  = (v_jrep[:, 128r:] max v_i) * g                    # DVE fused STT
    o[:, :128] diag <- true diagonal (copy_predicated)     # DVE, small
    DMA store o; PE-transpose the off-diag 128x128 sub-blocks, ACT-copy
    PSUM->SBUF, DMA store into the mirrored block.
where v = 2*relu(t_1), u = 2*relu(t_2) (relu(2*max(a,b)) == max(v_a,v_b)).
"""

import os
import sys
import types

import numpy as np

B, C, N, E = 2, 64, 1024, 32
NCORES = 8
EPC = B * E // NCORES  # 8 channels per core
P = 128
NB = N // P  # 8 row blocks

_CACHE = {}


def _ensure_hook_shim():
    """Make trace=True safe even when antenv.axon_hooks is absent."""
    try:
        import antenv.axon_hooks  # noqa: F401
    except ImportError:
        mod = types.ModuleType("antenv.axon_hooks")
        mod.get_axon_ntff_profile_hook = lambda: None
        mod.set_axon_ntff_profile_hook = lambda h: None
        sys.modules["antenv.axon_hooks"] = mod


def _build_program():
    import concourse.bacc as bacc
    import concourse.mybir as mybir
    import concourse.tile as tile

    dt = mybir.dt.float32
    AF = mybir.ActivationFunctionType
    ALU = mybir.AluOpType

    nc = bacc.Bacc("TRN2", target_bir_lowering=False, debug=False, num_devices=NCORES)

    emb = nc.declare_dram_parameter("emb", [C, N], dt, isOutput=False)
    w1t = nc.declare_dram_parameter("w1t", [C, EPC], dt, isOutput=False)
    w2t = nc.declare_dram_parameter("w2t", [C, EPC], dt, isOutput=False)
    th5c1 = nc.declare_dram_parameter("th5c1", [EPC, 1], dt, isOutput=False)
    th5c2 = nc.declare_dram_parameter("th5c2", [EPC, 1], dt, isOutput=False)
    eye = nc.declare_dram_parameter("eye", [P, P], dt, isOutput=False)
    out = nc.declare_dram_parameter("out", [EPC, N, N], dt, isOutput=True)

    H = N // 2  # matmul moving free-dim limit is 512

    with tile.TileContext(nc) as tc:
        with (
            tc.tile_pool(name="const", bufs=1) as cpool,
            tc.tile_pool(name="rows", bufs=1) as rpool,
        ):
            sb_emb = cpool.tile([C, N], dt)
            nc.sync.dma_start(out=sb_emb[:], in_=emb[:])
            sb_w1t = cpool.tile([C, EPC], dt)
            nc.sync.dma_start(out=sb_w1t[:], in_=w1t[:])
            sb_w2t = cpool.tile([C, EPC], dt)
            nc.sync.dma_start(out=sb_w2t[:], in_=w2t[:])
            sb_th5c1 = cpool.tile([EPC, 1], dt)
            nc.sync.dma_start(out=sb_th5c1[:], in_=th5c1[:])
            sb_th5c2 = cpool.tile([EPC, 1], dt)
            nc.sync.dma_start(out=sb_th5c2[:], in_=th5c2[:])
            sb_eye = cpool.tile([P, P], dt)
            nc.sync.dma_start(out=sb_eye[:], in_=eye[:])
            sb_ones = cpool.tile([1, P], dt)
            nc.vector.memset(sb_ones[:], 1.0)

            # Row-layout intermediates (channel on partition, node on free).
            sb_vrow = rpool.tile([EPC, N], dt)   # 2*relu(t1)
            sb_urow = rpool.tile([EPC, N], dt)   # 2*relu(t2)
            sb_dtrue = rpool.tile([EPC, N], dt)  # true diagonal values
            # Column layouts: [p, r*EPC + ch] = value at node r*128+p.
            sb_vcol = rpool.tile([P, NB * EPC], dt)
            sb_ucol = rpool.tile([P, NB * EPC], dt)
            sb_dcol = rpool.tile([P, NB * EPC], dt)

            with (
                tc.tile_pool(name="ph1ps", bufs=1, space="PSUM") as p1ps,
                tc.tile_pool(name="ph1sb", bufs=1) as p1sb,
            ):
                ps_t1 = p1ps.tile([EPC, N], dt)
                ps_t2 = p1ps.tile([EPC, N], dt)
                for h in range(2):
                    nc.tensor.matmul(
                        ps_t1[:, h * H:(h + 1) * H],
                        lhsT=sb_w1t[:],
                        rhs=sb_emb[:, h * H:(h + 1) * H],
                        start=True,
                        stop=True,
                    )
                    nc.tensor.matmul(
                        ps_t2[:, h * H:(h + 1) * H],
                        lhsT=sb_w2t[:],
                        rhs=sb_emb[:, h * H:(h + 1) * H],
                        start=True,
                        stop=True,
                    )
                nc.scalar.activation(sb_vrow[:], ps_t1[:], AF.Relu, scale=2.0)
                nc.scalar.activation(sb_urow[:], ps_t2[:], AF.Relu, scale=2.0)
                # True diagonal: relu(2t1+th5_1) * sigmoid(relu(2t2+th5_2))
                sb_d1 = p1sb.tile([EPC, N], dt)
                nc.scalar.activation(
                    sb_d1[:], ps_t1[:], AF.Relu, bias=sb_th5c1[:], scale=2.0
                )
                sb_d2 = p1sb.tile([EPC, N], dt)
                nc.scalar.activation(
                    sb_d2[:], ps_t2[:], AF.Relu, bias=sb_th5c2[:], scale=2.0
                )
                nc.scalar.activation(sb_d2[:], sb_d2[:], AF.Sigmoid)
                nc.vector.tensor_mul(sb_dtrue[:], sb_d1[:], sb_d2[:])

            with tc.tile_pool(name="colps", bufs=4, space="PSUM") as cps:
                for r in range(NB):
                    for src, dst in (
                        (sb_vrow, sb_vcol),
                        (sb_urow, sb_ucol),
                        (sb_dtrue, sb_dcol),
                    ):
                        pt_c = cps.tile([P, EPC], dt, tag="pt_c")
                        nc.tensor.transpose(
                            pt_c[:], src[:, r * P:(r + 1) * P], sb_eye[:EPC, :EPC]
                        )
                        nc.scalar.copy(dst[:, r * EPC:(r + 1) * EPC], pt_c[:])

            with (
                tc.tile_pool(name="jrepps", bufs=1, space="PSUM") as jps,
                tc.tile_pool(name="tpps", bufs=3, space="PSUM") as tps,
                tc.tile_pool(name="jrepsb", bufs=2) as jsb,
                tc.tile_pool(name="work", bufs=3) as wp,
                tc.tile_pool(name="tsb", bufs=4) as tsb,
            ):
                for ch in range(EPC):
                    # PE needs base partition 0 for both matmul operands;
                    # stage this channel's v/u row on partition 0 via DMA.
                    sb_vflat = jsb.tile([1, N], dt, tag="sb_vflat")
                    nc.sync.dma_start(out=sb_vflat[:], in_=sb_vrow[ch:ch + 1, :])
                    sb_uflat = jsb.tile([1, N], dt, tag="sb_uflat")
                    nc.sync.dma_start(out=sb_uflat[:], in_=sb_urow[ch:ch + 1, :])
                    ps_v = jps.tile([P, N], dt, tag="ps_v")
                    ps_u = jps.tile([P, N], dt, tag="ps_u")
                    for h in range(2):
                        nc.tensor.matmul(
                            ps_v[:, h * H:(h + 1) * H],
                            lhsT=sb_ones[:],
                            rhs=sb_vflat[:, h * H:(h + 1) * H],
                            start=True,
                            stop=True,
                        )
                        nc.tensor.matmul(
                            ps_u[:, h * H:(h + 1) * H],
                            lhsT=sb_ones[:],
                            rhs=sb_uflat[:, h * H:(h + 1) * H],
                            start=True,
                            stop=True,
                        )
                    sb_vj = jsb.tile([P, N], dt, tag="sb_vj")
                    nc.scalar.copy(sb_vj[:], ps_v[:])
                    sb_uj = jsb.tile([P, N], dt, tag="sb_uj")
                    nc.scalar.copy(sb_uj[:], ps_u[:])

                    for r in range(NB):
                        cb = r * P
                        W = N - cb
                        ci = r * EPC + ch
                        m2 = wp.tile([P, W], dt, tag="m2")
                        nc.vector.tensor_scalar_max(
                            m2[:], sb_uj[:, cb:], sb_ucol[:, ci:ci + 1]
                        )
                        g = wp.tile([P, W], dt, tag="g")
                        nc.scalar.activation(g[:], m2[:], AF.Sigmoid)
                        o = wp.tile([P, W], dt, tag="o")
                        nc.vector.scalar_tensor_tensor(
                            o[:],
                            sb_vj[:, cb:],
                            sb_vcol[:, ci:ci + 1],
                            g[:],
                            op0=ALU.max,
                            op1=ALU.mult,
                        )
                        nc.vector.copy_predicated(
                            o[:, :P],
                            sb_eye[:].bitcast(mybir.dt.int32),
                            sb_dcol[:, ci:ci + 1].broadcast_to([P, P]),
                        )
                        nc.sync.dma_start(
                            out=out[ch, cb:cb + P, cb:N], in_=o[:]
                        )
                        for s in range(r + 1, NB):
                            pt = tps.tile([P, P], dt, tag="pt")
                            nc.tensor.transpose(
                                pt[:], o[:, (s - r) * P:(s - r + 1) * P], sb_eye[:]
                            )
                            st = tsb.tile([P, P], dt, tag="st")
                            nc.scalar.copy(st[:], pt[:])
                            nc.sync.dma_start(
                                out=out[ch, s * P:(s + 1) * P, cb:cb + P], in_=st[:]
                            )

    nc.compile()
    return nc


def _get_program():
    if "nc" not in _CACHE:
        _CACHE["nc"] = _build_program()
    return _CACHE["nc"]


def kernel(**inputs):
    _ensure_hook_shim()
    from concourse.bass_utils import run_bass_kernel_spmd

    emb = np.ascontiguousarray(np.asarray(inputs["emb"], dtype=np.float32))
    th12_1 = np.asarray(inputs["th12_1"], dtype=np.float32)
    th12_2 = np.asarray(inputs["th12_2"], dtype=np.float32)
    th5_1 = np.asarray(inputs["th5_1"], dtype=np.float32)
    th5_2 = np.asarray(inputs["th5_2"], dtype=np.float32)
    eye = np.eye(P, dtype=np.float32)

    in_maps = []
    for k in range(NCORES):
        b = k // (NCORES // B)
        e0 = (k % (NCORES // B)) * EPC
        in_maps.append(
            {
                "emb": np.ascontiguousarray(emb[b]),
                "w1t": np.ascontiguousarray(th12_1[e0:e0 + EPC].T),
                "w2t": np.ascontiguousarray(th12_2[e0:e0 + EPC].T),
                "th5c1": np.ascontiguousarray(th5_1[e0:e0 + EPC, None]),
                "th5c2": np.ascontiguousarray(th5_2[e0:e0 + EPC, None]),
                "eye": eye,
            }
        )

    nc = _get_program()
    res = run_bass_kernel_spmd(nc, in_maps, core_ids=list(range(NCORES)))
    _CACHE["last_result"] = res

    out = np.empty((B, E, N, N), dtype=np.float32)
    for k in range(NCORES):
        b = k // (NCORES // B)
        e0 = (k % (NCORES // B)) * EPC
        out[b, e0:e0 + EPC] = res.results[k]["out"]
    return out
